# revision 6
# baseline (speedup 1.0000x reference)
"""Trainium2 kernel for nn_GravityHypothesisTester.

Heavy part (B x N x N distance matrices + row/col min/argmin) runs on 8
NeuronCores: core c handles (batch = c % 4, orientation = c // 4).
Orientation 0 reduces over tgt (rows = src points), orientation 1 reduces
over src (rows = tgt points).

The PE computes PSUM = -d^2 directly via a K=5 augmented float32r matmul
(rows [2a; aa; 1] x [b; -1; -bb] => 2 a.b - aa[n] - bb[m]).  float32r
truncates inputs to FP22 but runs 4x faster than fp32 (1 cycle/row at
512 moving columns).  Computing -d^2 (instead of 2 a.b - bb) makes the
interesting values (near the max, i.e. near the nearest neighbor) tiny,
so a subsequent fp16 quantization is gentle exactly where it matters.

Per [128,2048] half-tile: ACT drains PSUM to SBUF as fp16 (1 elem/cyc/lane,
dtype-independent); DVE pass 1 max-reduces the fp16 tile in 4x perf mode;
DVE pass 2 recovers the argmax as sum((vals==max)*iota) via
scalar_tensor_tensor in 2x perf mode (all-fp16 operands).

Host does the tiny O(B*N) pre/post work (Rodrigues, means, median, sigmoid)
plus exact-recovery: matched distances are recomputed in fp32 from the
returned indices, and rows whose recovered distance disagrees with the
device max value (fp16 tie-sum artifacts, ~tens of rows) are re-argmin'd
exactly on host.  Simulated end-to-end rel-l2 error: 1.7e-4.
"""

import sys
from contextlib import ExitStack

import numpy as np

sys.path.insert(0, "/opt/trn_rl_repo")

import concourse.bass as bass
import concourse.tile as tile
from concourse import bacc, mybir
from concourse.bass_utils import run_bass_kernel_spmd  # noqa: F401  (kept for harness compat)

EPS = 1e-6
CHI2_THRESH = 9.0
DIST_SCALE = 3.0
B = 4
N = 4096
P = 128
K = 5                       # 3 coords + aa + ones rows
ROW_TILES = N // P          # 32
HALF = 2048                 # half-tile free size (4 PSUM banks)
MMCOLS = 512                # moving-operand max
N_CORES = 8
NHALF = ROW_TILES * 2       # 64

_NC_CACHE = {}


def _build_nc(repeat=1):
    """Build the SPMD bass program (identical on all 8 cores)."""
    key = ("nc", repeat)
    if key in _NC_CACHE:
        return _NC_CACHE[key]

    nc = bacc.Bacc("TRN2", target_bir_lowering=False)
    f32 = mybir.dt.float32
    f16 = mybir.dt.float16
    f32r = mybir.dt.float32r

    lhsT_d = nc.dram_tensor("lhsT", [K, N], f32r, kind="ExternalInput")
    rhs_d = nc.dram_tensor("rhs", [K, N], f32r, kind="ExternalInput")
    vals_d = nc.dram_tensor("vals", [P, NHALF], f32, kind="ExternalOutput")
    idxs_d = nc.dram_tensor("idxs", [P, NHALF], f32, kind="ExternalOutput")

    with tile.TileContext(nc) as tc, ExitStack() as ctx:
        inp = ctx.enter_context(tc.tile_pool(name="inp", bufs=1))
        stage = ctx.enter_context(tc.tile_pool(name="stage", bufs=1))
        cpool = ctx.enter_context(tc.tile_pool(name="cpool", bufs=3))
        psum = ctx.enter_context(tc.tile_pool(name="psum", bufs=2, space="PSUM"))

        lhsT_s = inp.tile([K, N], f32r)
        rhs_s = inp.tile([K, N], f32r)
        nc.sync.dma_start(lhsT_s[:], lhsT_d[:])
        nc.sync.dma_start(rhs_s[:], rhs_d[:])

        vstage = stage.tile([P, NHALF], f32)
        istage = stage.tile([P, NHALF], f32)
        scratch16 = stage.tile([P, HALF], f16)
        iota_i = stage.tile([P, HALF], mybir.dt.int32)
        iota16 = stage.tile([P, HALF], f16)
        nc.gpsimd.iota(iota_i[:], pattern=[[1, HALF]], base=0,
                       channel_multiplier=0)
        nc.vector.tensor_copy(iota16[:], iota_i[:])

        for t in range(ROW_TILES * repeat):
            rt = t % ROW_TILES
            for h in range(2):
                i = rt * 2 + h
                pt = psum.tile([P, HALF], f32)
                if t == 0 and h == 0:
                    # Dummy matmul reading only rhs_s: the PE weight-load HW
                    # slot carries a single semaphore wait, so the two input
                    # DMA waits must land on separate PE instructions.
                    nc.tensor.matmul(
                        pt[:, 0:MMCOLS], rhs_s[:, 0:P], rhs_s[:, 0:MMCOLS],
                        start=True, stop=True,
                    )
                for j in range(HALF // MMCOLS):
                    nc.tensor.matmul(
                        pt[:, bass.ts(j, MMCOLS)],
                        lhsT_s[:, bass.ts(rt, P)],
                        rhs_s[:, h * HALF + j * MMCOLS
                              : h * HALF + (j + 1) * MMCOLS],
                        start=True,
                        stop=True,
                    )
                v1 = vstage[:, i : i + 1]
                i1 = istage[:, i : i + 1]
                # ACT drains PSUM (fp32 -> fp16); DVE then reduces pure-SBUF
                # fp16 operands at 4x (max) / 2x (match) perf modes.
                sb16 = cpool.tile([P, HALF], f16)
                nc.scalar.copy(sb16[:], pt[:])
                nc.vector.tensor_scalar(
                    scratch16[:], sb16[:], 0.0, None,
                    op0=mybir.AluOpType.add, op1=mybir.AluOpType.max,
                    accum_out=v1,
                )
                # scratch = (sb16 == v1) * iota; i1 = sum(scratch)
                nc.vector.scalar_tensor_tensor(
                    scratch16[:], sb16[:], v1, iota16[:],
                    op0=mybir.AluOpType.is_equal, op1=mybir.AluOpType.mult,
                    accum_out=i1,
                )

        nc.sync.dma_start(vals_d[:], vstage[:])
        nc.sync.dma_start(idxs_d[:], istage[:])

    nc.finalize()
    _NC_CACHE[key] = nc
    return nc


def _get_runner(repeat=1):
    """Build the sharded PJRT executable once; reuse across kernel() calls."""
    rkey = ("runner", repeat)
    if rkey in _NC_CACHE:
        return _NC_CACHE[rkey]

    import jax
    from jax.sharding import Mesh, PartitionSpec
    from jax.experimental.shard_map import shard_map
    from concourse import bass2jax

    nc = _build_nc(repeat)
    bass2jax.install_neuronx_cc_hook()

    partition_name = nc.partition_id_tensor.name if nc.partition_id_tensor else None
    in_names, out_names, out_avals, zero_outs = [], [], [], []
    for alloc in nc.m.functions[0].allocations:
        if not isinstance(alloc, mybir.MemoryLocationSet):
            continue
        name = alloc.memorylocations[0].name
        if alloc.kind == "ExternalInput":
            if name != partition_name:
                in_names.append(name)
        elif alloc.kind == "ExternalOutput":
            shape = tuple(alloc.tensor_shape)
            np_dt = mybir.dt.np(alloc.dtype)
            out_names.append(name)
            out_avals.append(jax.core.ShapedArray(shape, np_dt))
            zero_outs.append(np.zeros(shape, np_dt))

    n_params = len(in_names)
    n_outs = len(out_names)
    all_in_names = list(in_names) + list(out_names)
    if partition_name is not None:
        all_in_names.append(partition_name)
    donate = tuple(range(n_params, n_params + n_outs))

    def _body(*args):
        operands = list(args)
        if partition_name is not None:
            operands.append(bass2jax.partition_id_tensor())
        outs = bass2jax._bass_exec_p.bind(
            *operands,
            out_avals=tuple(out_avals),
            in_names=tuple(all_in_names),
            out_names=tuple(out_names),
            lowering_input_output_aliases=(),
            sim_require_finite=True,
            sim_require_nnan=True,
            nc=nc,
        )
        return tuple(outs)

    devices = jax.devices()[:N_CORES]
    mesh = Mesh(np.asarray(devices), ("core",))
    in_specs = (PartitionSpec("core"),) * (n_params + n_outs)
    out_specs = (PartitionSpec("core"),) * n_outs
    sharded = jax.jit(
        shard_map(_body, mesh=mesh, in_specs=in_specs, out_specs=out_specs,
                  check_rep=False),
        donate_argnums=donate, keep_unused=True,
    )

    def run(in_maps):
        concat_in = [
            np.concatenate([np.asarray(m[name]) for m in in_maps], axis=0)
            for name in in_names
        ]
        concat_zeros = [
            np.zeros((N_CORES * z.shape[0], *z.shape[1:]), z.dtype)
            for z in zero_outs
        ]
        out_arrs = sharded(*concat_in, *concat_zeros)
        return [
            {
                name: np.asarray(out_arrs[k]).reshape(
                    N_CORES, *out_avals[k].shape)[c]
                for k, name in enumerate(out_names)
            }
            for c in range(N_CORES)
        ]

    _NC_CACHE[rkey] = run
    return run


# ---------------- host-side numpy port of the tiny reference pieces ----------


def _normalize(x, axis, eps=EPS):
    n = np.linalg.norm(x, axis=axis, keepdims=True)
    return x / np.maximum(n, eps)


def _skew(k):
    kx, ky, kz = k[:, 0], k[:, 1], k[:, 2]
    O = np.zeros_like(kx)
    row0 = np.stack([O, -kz, ky], axis=1)
    row1 = np.stack([kz, O, -kx], axis=1)
    row2 = np.stack([-ky, kx, O], axis=1)
    return np.stack([row0, row1, row2], axis=1)


def _gravity_align(g_src, g_tgt, eps=EPS):
    u = _normalize(g_src, 1, eps)
    v = _normalize(g_tgt, 1, eps)
    axis = np.cross(u, v)
    axis_norm = np.linalg.norm(axis, axis=1, keepdims=True)
    dot = np.clip(np.sum(u * v, axis=1, keepdims=True), -1.0, 1.0)
    parallel = axis_norm < 1e-6
    k = axis / (axis_norm + eps)
    theta = np.arccos(dot)
    sin_t, cos_t = np.sin(theta), np.cos(theta)
    Kk = _skew(k)
    I = np.eye(3, dtype=g_src.dtype)[None]
    R = I + sin_t[:, :, None] * Kk + (1.0 - cos_t)[:, :, None] * (Kk @ Kk)
    ex = np.array([1.0, 0.0, 0.0], dtype=u.dtype)[None]
    ey = np.array([0.0, 1.0, 0.0], dtype=u.dtype)[None]
    use_ex = np.abs(u[:, 0:1]) < 0.9
    basis = np.where(use_ex, ex, ey)
    axis2 = _normalize(np.cross(u, basis), 1, eps)
    K2 = _skew(axis2)
    R_anti = I + 2.0 * (K2 @ K2)
    antipar = parallel & (dot < 0.0)
    R = np.where(antipar[:, :, None], R_anti, R)
    R = np.where((parallel & (dot > 0.0))[:, :, None], I, R)
    return R.astype(np.float32)


def _decode_core(vals, idxs):
    """[P, 64] staging -> (maxval[4096], argidx[4096]) for one core.

    Row r = 128*t + p lives at partition p, column 2t+h.  idx may be a
    tie-sum artifact (out of range); caller detects and repairs.
    """
    v = vals.reshape(P, ROW_TILES, 2)
    i = np.rint(idxs.reshape(P, ROW_TILES, 2)).astype(np.int64)
    v0, v1 = v[:, :, 0], v[:, :, 1]
    i0, i1 = i[:, :, 0], i[:, :, 1]
    take1 = v1 > v0  # strict: ties -> first occurrence (half 0)
    vmax = np.where(take1, v1, v0)          # [128, 32]
    idx = np.where(take1, i1 + HALF, i0)    # [128, 32]
    return vmax.T.reshape(N), idx.T.reshape(N)


def _sigmoid(x):
    out = np.empty_like(x)
    pos = x >= 0
    out[pos] = 1.0 / (1.0 + np.exp(-x[pos]))
    ex = np.exp(x[~pos])
    out[~pos] = ex / (1.0 + ex)
    return out


def _host_prep(src, tgt, src_n, tgt_n, g_p, k_p, g_q, k_q):
    src = np.asarray(src, np.float32)
    tgt = np.asarray(tgt, np.float32)
    src_n = np.asarray(src_n, np.float32)
    g_p = np.asarray(g_p, np.float32)
    g_q = np.asarray(g_q, np.float32)

    R_g = _gravity_align(g_p, g_q)
    src_rot = np.einsum("bij,bjn->bin", R_g, src).astype(np.float32)
    src_n_rot = np.einsum("bij,bjn->bin", R_g, src_n).astype(np.float32)
    t_center = tgt.mean(axis=2, keepdims=True) - src_rot.mean(axis=2, keepdims=True)
    s = (src_rot + t_center).astype(np.float32)  # src_init

    xx = np.sum(s * s, axis=1)  # [B, N]
    yy = np.sum(tgt * tgt, axis=1)
    return s, tgt, src_n_rot, xx, yy


def prepare_in_maps(src, tgt, src_n, tgt_n, g_p, k_p, g_q, k_q):
    s, tgt, _, xx, yy = _host_prep(src, tgt, src_n, tgt_n, g_p, k_p, g_q, k_q)
    ones = np.ones((1, N), np.float32)
    in_maps = []
    for c in range(N_CORES):
        b, o = c % B, c // B
        if o == 0:
            lhsT = np.concatenate([2.0 * s[b], xx[b][None], ones], axis=0)
            rhs = np.concatenate([tgt[b], -ones, -yy[b][None]], axis=0)
        else:
            lhsT = np.concatenate([2.0 * tgt[b], yy[b][None], ones], axis=0)
            rhs = np.concatenate([s[b], -ones, -xx[b][None]], axis=0)
        in_maps.append(
            {"lhsT": np.ascontiguousarray(lhsT, np.float32),
             "rhs": np.ascontiguousarray(rhs, np.float32)}
        )
    return in_maps


def _recover_and_repair(A, Bm, idx_raw, dev):
    """Exact matched distances for query points A [3,N] against Bm [3,N].

    idx_raw: device argmax indices (may contain tie-sum garbage).
    dev: device max values (= -d^2, fp16-quantized, fp22 matmul noise).
    Returns (min_d2 [N], corr [N]) with bad rows re-argmin'd exactly.
    """
    idx = np.clip(idx_raw, 0, N - 1)
    rec = ((A - Bm[:, idx]) ** 2).sum(axis=0)
    tol = 5e-3 + 5e-3 * np.abs(dev)
    bad = (idx_raw != idx) | (np.abs(rec + dev) > tol)
    if bad.any():
        rows = np.nonzero(bad)[0]
        d2 = ((A[:, rows][:, :, None] - Bm[:, None, :]) ** 2).sum(axis=0)
        idx[rows] = d2.argmin(axis=1)
        rec[rows] = d2.min(axis=1)
    return rec.astype(np.float32), idx


def kernel(src, tgt, src_n, tgt_n, g_p, k_p, g_q, k_q):
    src = np.asarray(src, np.float32)
    tgt = np.asarray(tgt, np.float32)
    src_n = np.asarray(src_n, np.float32)
    tgt_n = np.asarray(tgt_n, np.float32)
    g_p = np.asarray(g_p, np.float32)
    g_q = np.asarray(g_q, np.float32)
    k_p = np.asarray(k_p, np.float32)
    k_q = np.asarray(k_q, np.float32)

    s, tgt, src_n_rot, xx, yy = _host_prep(
        src, tgt, src_n, tgt_n, g_p, k_p, g_q, k_q)
    in_maps = prepare_in_maps(src, tgt, src_n, tgt_n, g_p, k_p, g_q, k_q)

    results = _get_runner()(in_maps)

    min_pq = np.empty((B, N), np.float32)
    corr_p2q = np.empty((B, N), np.int64)
    min_qp = np.empty((B, N), np.float32)
    corr_q2p = np.empty((B, N), np.int64)
    for c in range(N_CORES):
        b, o = c % B, c // B
        vmax, idx = _decode_core(results[c]["vals"], results[c]["idxs"])
        if o == 0:
            min_pq[b], corr_p2q[b] = _recover_and_repair(s[b], tgt[b], idx, vmax)
        else:
            min_qp[b], corr_q2p[b] = _recover_and_repair(tgt[b], s[b], idx, vmax)

    nn_d_p = np.sqrt(np.maximum(min_pq, 0.0) + EPS)
    nn_d_q = np.sqrt(np.maximum(min_qp, 0.0) + EPS)
    tau_p = DIST_SCALE * np.sort(nn_d_p, axis=1)[:, (N - 1) // 2][:, None]
    tau_q = DIST_SCALE * np.sort(nn_d_q, axis=1)[:, (N - 1) // 2][:, None]
    geom_p = (nn_d_p <= tau_p).astype(np.float32)
    geom_q = (nn_d_q <= tau_q).astype(np.float32)

    gq = g_q[:, :, None]
    inc_p = np.sum(src_n_rot * gq, axis=1)  # [B, N]
    inc_q = np.sum(tgt_n * gq, axis=1)
    inc_p_ref = np.take_along_axis(inc_q, corr_p2q, axis=1)
    inc_q_ref = np.take_along_axis(inc_p, corr_q2p, axis=1)

    k_eff = k_p * k_q / (k_p + k_q + EPS)  # [B,1]
    w_p = _sigmoid(CHI2_THRESH - k_eff * (inc_p - inc_p_ref) ** 2) * geom_p
    w_q = _sigmoid(CHI2_THRESH - k_eff * (inc_q - inc_q_ref) ** 2) * geom_q
    return w_p[:, None, :].astype(np.float32), w_q[:, None, :].astype(np.float32)


# revision 12
# speedup vs baseline: 1.2060x; 1.2060x over previous
"""Trainium2 kernel for nn_GravityHypothesisTester.

Heavy part (B x N x N distance matrices + row/col min/argmin) runs on 8
NeuronCores: core c handles (batch = c % 4, orientation = c // 4).
Orientation 0 reduces over tgt (rows = src points), orientation 1 reduces
over src (rows = tgt points).

The PE computes PSUM = -d^2 directly via a K=5 augmented float32r matmul
(rows [2a; aa; 1] x [b; -1; -bb] => 2 a.b - aa[n] - bb[m]).  float32r
truncates inputs to FP22 but streams 4x faster than fp32 (1 cycle/row at
512 moving columns).

The row argmax of each [128, 2048] PSUM half-tile is computed by a single
custom DVE instruction (ARGMAX_LAST_ANT, registered below): the body
`select(Src0 >= running_max(Src0), Idx, -FLT_MAX)` with a MAX-accumulate
yields the index of the (last) row maximum in ONE 1-elem/cycle pass read
directly from PSUM.  This replaces the ACT PSUM->SBUF drain, the DVE
max-reduce, and the DVE match pass of the classic 3-pass scheme, and has
no fp16 quantization or tie-sum artifacts.

Host does the tiny O(B*N) pre/post work (Rodrigues, means, median,
sigmoid): it gathers the two half-candidates per row, recomputes their
exact fp32 distances, and keeps the smaller — so distances entering the
median/threshold logic are exact regardless of device quantization.
End-to-end rel-l2 error vs the fp32 oracle: ~1.7e-4 (fp22 argmin flips
among near-ties only).
"""

import sys
from contextlib import ExitStack

import numpy as np

sys.path.insert(0, "/opt/trn_rl_repo")

import concourse.bass as bass
import concourse.tile as tile
from concourse import bacc, dve_ops, mybir
from concourse.bass_utils import run_bass_kernel_spmd  # noqa: F401  (harness compat)
from concourse.dve_ops import DveOp
from concourse.dve_spec import AluOp, Idx, MaxNeg, Spec, Src0, lower, scan, select
from concourse.dve_uop import DveOpSpec

EPS = 1e-6
CHI2_THRESH = 9.0
DIST_SCALE = 3.0
B = 4
N = 4096
P = 128
K = 5                       # 3 coords + aa + ones rows
ROW_TILES = N // P          # 32
HALF = 2048                 # half-tile free size (4 PSUM banks)
MMCOLS = 512                # moving-operand max
N_CORES = 8
NHALF = ROW_TILES * 2       # 64

_NC_CACHE = {}


def _argmax_ref(in0, in1, c0, c1, c2):
    r = np.maximum.accumulate(in0.astype(np.float32), axis=1)
    idx = np.arange(in0.shape[1], dtype=np.float32)[None, :]
    out = np.where(in0.astype(np.float32) >= r, idx, -np.finfo(np.float32).max)
    acc = out.max(axis=1, keepdims=True)
    return out, acc


def _get_argmax_op():
    """Register the fused argmax custom DVE op (idempotent)."""
    name = "ARGMAX_LAST_ANT"
    for op in dve_ops.OPS:
        if op.name == name:
            return op
    spec = Spec(
        body=select(Src0 >= scan(AluOp.MAX, Src0), Idx, MaxNeg),
        accum=AluOp.MAX,
        reference=_argmax_ref,
    )
    row = dve_ops._CUSTOM_DVE_ROW_BASE + len(dve_ops.OPS)
    dve_ops._SUB_OPCODE_FOR_NAME[name] = row
    shas = {
        ver: DveOpSpec(name=name, opcode=row, uops=lower(spec, ver=ver),
                       rd1_en=False).sha(ver)
        for ver in ("v3", "v4")
    }
    op = DveOp(name, spec, subdim=False, uops_sha=shas)
    dve_ops.OPS.append(op)
    dve_ops.CUSTOM_DVE_SPECS[name] = spec
    return op


def _build_nc(repeat=1):
    """Build the SPMD bass program (identical on all 8 cores)."""
    key = ("nc", repeat)
    if key in _NC_CACHE:
        return _NC_CACHE[key]

    argmax_op = _get_argmax_op()

    nc = bacc.Bacc("TRN2", target_bir_lowering=False)
    f32 = mybir.dt.float32
    f32r = mybir.dt.float32r

    lhsT_d = nc.dram_tensor("lhsT", [K, N], f32r, kind="ExternalInput")
    rhs_d = nc.dram_tensor("rhs", [K, N], f32r, kind="ExternalInput")
    idxs_d = nc.dram_tensor("idxs", [P, NHALF], f32, kind="ExternalOutput")

    with tile.TileContext(nc) as tc, ExitStack() as ctx:
        inp = ctx.enter_context(tc.tile_pool(name="inp", bufs=1))
        stage = ctx.enter_context(tc.tile_pool(name="stage", bufs=1))
        psum = ctx.enter_context(tc.tile_pool(name="psum", bufs=2, space="PSUM"))

        lhsT_s = inp.tile([K, N], f32r)
        rhs_s = inp.tile([K, N], f32r)
        nc.sync.dma_start(lhsT_s[:], lhsT_d[:])
        nc.sync.dma_start(rhs_s[:], rhs_d[:])

        istage = stage.tile([P, NHALF], f32)
        scratch = stage.tile([P, HALF], f32)

        for t in range(ROW_TILES * repeat):
            rt = t % ROW_TILES
            for h in range(2):
                i = rt * 2 + h
                pt = psum.tile([P, HALF], f32)
                if t == 0 and h == 0:
                    # Dummy matmul reading only rhs_s: the PE weight-load HW
                    # slot carries a single semaphore wait, so the two input
                    # DMA waits must land on separate PE instructions.
                    nc.tensor.matmul(
                        pt[:, 0:MMCOLS], rhs_s[:, 0:P], rhs_s[:, 0:MMCOLS],
                        start=True, stop=True,
                    )
                for j in range(HALF // MMCOLS):
                    nc.tensor.matmul(
                        pt[:, bass.ts(j, MMCOLS)],
                        lhsT_s[:, bass.ts(rt, P)],
                        rhs_s[:, h * HALF + j * MMCOLS
                              : h * HALF + (j + 1) * MMCOLS],
                        start=True,
                        stop=True,
                    )
                # One fused pass straight from PSUM:
                #   istage[:, i] = argmax_m(pt[:, m])  (last occurrence)
                nc.vector._custom_dve(
                    argmax_op, out=scratch[:], in0=pt[:],
                    accum_out=istage[:, i : i + 1],
                )

        nc.sync.dma_start(idxs_d[:], istage[:])

    nc.finalize()
    _NC_CACHE[key] = nc
    return nc


def _get_runner(repeat=1):
    """Build the sharded PJRT executable once; reuse across kernel() calls."""
    rkey = ("runner", repeat)
    if rkey in _NC_CACHE:
        return _NC_CACHE[rkey]
    run = _make_runner_from_nc(_build_nc(repeat))
    _NC_CACHE[rkey] = run
    return run


def _make_runner_from_nc(nc):
    import jax
    from jax.sharding import Mesh, PartitionSpec
    from jax.experimental.shard_map import shard_map
    from concourse import bass2jax

    bass2jax.install_neuronx_cc_hook()

    partition_name = nc.partition_id_tensor.name if nc.partition_id_tensor else None
    in_names, out_names, out_avals, zero_outs = [], [], [], []
    for alloc in nc.m.functions[0].allocations:
        if not isinstance(alloc, mybir.MemoryLocationSet):
            continue
        name = alloc.memorylocations[0].name
        if alloc.kind == "ExternalInput":
            if name != partition_name:
                in_names.append(name)
        elif alloc.kind == "ExternalOutput":
            shape = tuple(alloc.tensor_shape)
            np_dt = mybir.dt.np(alloc.dtype)
            out_names.append(name)
            out_avals.append(jax.core.ShapedArray(shape, np_dt))
            zero_outs.append(np.zeros(shape, np_dt))

    n_params = len(in_names)
    n_outs = len(out_names)
    all_in_names = list(in_names) + list(out_names)
    if partition_name is not None:
        all_in_names.append(partition_name)
    donate = tuple(range(n_params, n_params + n_outs))

    def _body(*args):
        operands = list(args)
        if partition_name is not None:
            operands.append(bass2jax.partition_id_tensor())
        outs = bass2jax._bass_exec_p.bind(
            *operands,
            out_avals=tuple(out_avals),
            in_names=tuple(all_in_names),
            out_names=tuple(out_names),
            lowering_input_output_aliases=(),
            sim_require_finite=True,
            sim_require_nnan=True,
            nc=nc,
        )
        return tuple(outs)

    devices = jax.devices()[:N_CORES]
    mesh = Mesh(np.asarray(devices), ("core",))
    in_specs = (PartitionSpec("core"),) * (n_params + n_outs)
    out_specs = (PartitionSpec("core"),) * n_outs
    sharded = jax.jit(
        shard_map(_body, mesh=mesh, in_specs=in_specs, out_specs=out_specs,
                  check_rep=False),
        donate_argnums=donate, keep_unused=True,
    )

    def run(in_maps):
        concat_in = [
            np.concatenate([np.asarray(m[name]) for m in in_maps], axis=0)
            for name in in_names
        ]
        concat_zeros = [
            np.zeros((N_CORES * z.shape[0], *z.shape[1:]), z.dtype)
            for z in zero_outs
        ]
        out_arrs = sharded(*concat_in, *concat_zeros)
        return [
            {
                name: np.asarray(out_arrs[k]).reshape(
                    N_CORES, *out_avals[k].shape)[c]
                for k, name in enumerate(out_names)
            }
            for c in range(N_CORES)
        ]

    return run


# ---------------- host-side numpy port of the tiny reference pieces ----------


def _normalize(x, axis, eps=EPS):
    n = np.linalg.norm(x, axis=axis, keepdims=True)
    return x / np.maximum(n, eps)


def _skew(k):
    kx, ky, kz = k[:, 0], k[:, 1], k[:, 2]
    O = np.zeros_like(kx)
    row0 = np.stack([O, -kz, ky], axis=1)
    row1 = np.stack([kz, O, -kx], axis=1)
    row2 = np.stack([-ky, kx, O], axis=1)
    return np.stack([row0, row1, row2], axis=1)


def _gravity_align(g_src, g_tgt, eps=EPS):
    u = _normalize(g_src, 1, eps)
    v = _normalize(g_tgt, 1, eps)
    axis = np.cross(u, v)
    axis_norm = np.linalg.norm(axis, axis=1, keepdims=True)
    dot = np.clip(np.sum(u * v, axis=1, keepdims=True), -1.0, 1.0)
    parallel = axis_norm < 1e-6
    k = axis / (axis_norm + eps)
    theta = np.arccos(dot)
    sin_t, cos_t = np.sin(theta), np.cos(theta)
    Kk = _skew(k)
    I = np.eye(3, dtype=g_src.dtype)[None]
    R = I + sin_t[:, :, None] * Kk + (1.0 - cos_t)[:, :, None] * (Kk @ Kk)
    ex = np.array([1.0, 0.0, 0.0], dtype=u.dtype)[None]
    ey = np.array([0.0, 1.0, 0.0], dtype=u.dtype)[None]
    use_ex = np.abs(u[:, 0:1]) < 0.9
    basis = np.where(use_ex, ex, ey)
    axis2 = _normalize(np.cross(u, basis), 1, eps)
    K2 = _skew(axis2)
    R_anti = I + 2.0 * (K2 @ K2)
    antipar = parallel & (dot < 0.0)
    R = np.where(antipar[:, :, None], R_anti, R)
    R = np.where((parallel & (dot > 0.0))[:, :, None], I, R)
    return R.astype(np.float32)


def _decode_core(idxs):
    """[P, 64] staging -> (idx0[4096], idx1[4096]) candidate indices.

    Row r = 128*t + p lives at partition p, column 2t+h.  idx1 is relative
    to the second half (add HALF for the global column).
    """
    i = np.rint(idxs.reshape(P, ROW_TILES, 2)).astype(np.int64)
    i0 = np.clip(i[:, :, 0], 0, HALF - 1)
    i1 = np.clip(i[:, :, 1], 0, HALF - 1) + HALF
    return i0.T.reshape(N), i1.T.reshape(N)


def _sigmoid(x):
    out = np.empty_like(x)
    pos = x >= 0
    out[pos] = 1.0 / (1.0 + np.exp(-x[pos]))
    ex = np.exp(x[~pos])
    out[~pos] = ex / (1.0 + ex)
    return out


def _host_prep(src, tgt, src_n, tgt_n, g_p, k_p, g_q, k_q):
    src = np.asarray(src, np.float32)
    tgt = np.asarray(tgt, np.float32)
    src_n = np.asarray(src_n, np.float32)
    g_p = np.asarray(g_p, np.float32)
    g_q = np.asarray(g_q, np.float32)

    R_g = _gravity_align(g_p, g_q)
    src_rot = np.einsum("bij,bjn->bin", R_g, src).astype(np.float32)
    src_n_rot = np.einsum("bij,bjn->bin", R_g, src_n).astype(np.float32)
    t_center = tgt.mean(axis=2, keepdims=True) - src_rot.mean(axis=2, keepdims=True)
    s = (src_rot + t_center).astype(np.float32)  # src_init

    xx = np.sum(s * s, axis=1)  # [B, N]
    yy = np.sum(tgt * tgt, axis=1)
    return s, tgt, src_n_rot, xx, yy


def prepare_in_maps(src, tgt, src_n, tgt_n, g_p, k_p, g_q, k_q):
    s, tgt, _, xx, yy = _host_prep(src, tgt, src_n, tgt_n, g_p, k_p, g_q, k_q)
    ones = np.ones((1, N), np.float32)
    in_maps = []
    for c in range(N_CORES):
        b, o = c % B, c // B
        if o == 0:
            lhsT = np.concatenate([2.0 * s[b], xx[b][None], ones], axis=0)
            rhs = np.concatenate([tgt[b], -ones, -yy[b][None]], axis=0)
        else:
            lhsT = np.concatenate([2.0 * tgt[b], yy[b][None], ones], axis=0)
            rhs = np.concatenate([s[b], -ones, -xx[b][None]], axis=0)
        in_maps.append(
            {"lhsT": np.ascontiguousarray(lhsT, np.float32),
             "rhs": np.ascontiguousarray(rhs, np.float32)}
        )
    return in_maps


def _combine_halves(A, Bm, idx0, idx1):
    """Pick the better of the two half-candidates using exact distances.

    A [3,N] query points, Bm [3,N] reference points; idx0/idx1 [N] global
    candidate indices.  Returns (min_d2 [N], corr [N]).
    """
    d0 = ((A - Bm[:, idx0]) ** 2).sum(axis=0)
    d1 = ((A - Bm[:, idx1]) ** 2).sum(axis=0)
    take1 = d1 < d0
    return np.where(take1, d1, d0).astype(np.float32), np.where(take1, idx1, idx0)


def kernel(src, tgt, src_n, tgt_n, g_p, k_p, g_q, k_q):
    src = np.asarray(src, np.float32)
    tgt = np.asarray(tgt, np.float32)
    src_n = np.asarray(src_n, np.float32)
    tgt_n = np.asarray(tgt_n, np.float32)
    g_p = np.asarray(g_p, np.float32)
    g_q = np.asarray(g_q, np.float32)
    k_p = np.asarray(k_p, np.float32)
    k_q = np.asarray(k_q, np.float32)

    s, tgt, src_n_rot, xx, yy = _host_prep(
        src, tgt, src_n, tgt_n, g_p, k_p, g_q, k_q)
    in_maps = prepare_in_maps(src, tgt, src_n, tgt_n, g_p, k_p, g_q, k_q)

    results = _get_runner()(in_maps)

    min_pq = np.empty((B, N), np.float32)
    corr_p2q = np.empty((B, N), np.int64)
    min_qp = np.empty((B, N), np.float32)
    corr_q2p = np.empty((B, N), np.int64)
    for c in range(N_CORES):
        b, o = c % B, c // B
        idx0, idx1 = _decode_core(results[c]["idxs"])
        if o == 0:
            min_pq[b], corr_p2q[b] = _combine_halves(s[b], tgt[b], idx0, idx1)
        else:
            min_qp[b], corr_q2p[b] = _combine_halves(tgt[b], s[b], idx0, idx1)

    nn_d_p = np.sqrt(np.maximum(min_pq, 0.0) + EPS)
    nn_d_q = np.sqrt(np.maximum(min_qp, 0.0) + EPS)
    tau_p = DIST_SCALE * np.sort(nn_d_p, axis=1)[:, (N - 1) // 2][:, None]
    tau_q = DIST_SCALE * np.sort(nn_d_q, axis=1)[:, (N - 1) // 2][:, None]
    geom_p = (nn_d_p <= tau_p).astype(np.float32)
    geom_q = (nn_d_q <= tau_q).astype(np.float32)

    gq = g_q[:, :, None]
    inc_p = np.sum(src_n_rot * gq, axis=1)  # [B, N]
    inc_q = np.sum(tgt_n * gq, axis=1)
    inc_p_ref = np.take_along_axis(inc_q, corr_p2q, axis=1)
    inc_q_ref = np.take_along_axis(inc_p, corr_q2p, axis=1)

    k_eff = k_p * k_q / (k_p + k_q + EPS)  # [B,1]
    w_p = _sigmoid(CHI2_THRESH - k_eff * (inc_p - inc_p_ref) ** 2) * geom_p
    w_q = _sigmoid(CHI2_THRESH - k_eff * (inc_q - inc_q_ref) ** 2) * geom_q
    return w_p[:, None, :].astype(np.float32), w_q[:, None, :].astype(np.float32)


# revision 16
# speedup vs baseline: 2.2790x; 1.8897x over previous
"""Trainium2 kernel for nn_GravityHypothesisTester.

Heavy part (B x N x N distance matrices + row/col min/argmin) runs on 8
NeuronCores: core c handles (batch = c % 4, orientation = c // 4).
Orientation 0 reduces over tgt (rows = src points), orientation 1 reduces
over src (rows = tgt points).

The PE computes PSUM = -d^2 directly via a K=5 augmented float32r matmul
(rows [2a; aa; 1] x [b; -1; -bb] => 2 a.b - aa[n] - bb[m]).  float32r
truncates inputs to FP22 but streams 4x faster than fp32 (1 cycle/row at
512 moving columns).

The row argmax of each [128, 2048] PSUM half-tile is computed by a single
custom DVE instruction (ARGMAX_LAST_ANT, registered below): the body
`select(Src0 >= running_max(Src0), Idx, -FLT_MAX)` with a MAX-accumulate
yields the index of the (last) row maximum in ONE 1-elem/cycle pass read
directly from PSUM.  This replaces the ACT PSUM->SBUF drain, the DVE
max-reduce, and the DVE match pass of the classic 3-pass scheme, and has
no fp16 quantization or tie-sum artifacts.

Host does the tiny O(B*N) pre/post work (Rodrigues, means, median,
sigmoid): it gathers the two half-candidates per row, recomputes their
exact fp32 distances, and keeps the smaller — so distances entering the
median/threshold logic are exact regardless of device quantization.
End-to-end rel-l2 error vs the fp32 oracle: ~1.7e-4 (fp22 argmin flips
among near-ties only).
"""

import sys
from contextlib import ExitStack

import numpy as np

sys.path.insert(0, "/opt/trn_rl_repo")

import concourse.bass as bass
import concourse.tile as tile
from concourse import bacc, dve_ops, mybir
from concourse.bass_utils import run_bass_kernel_spmd  # noqa: F401  (harness compat)
from concourse.dve_ops import DveOp
from concourse.dve_spec import AluOp, Idx, MaxNeg, Spec, Src0, lower, scan, select
from concourse.dve_uop import DveOpSpec

EPS = 1e-6
CHI2_THRESH = 9.0
DIST_SCALE = 3.0
B = 4
N = 4096
P = 128
K = 5                       # 3 coords + aa + ones rows
ROW_TILES = N // P          # 32
HALF = 2048                 # half-tile free size (4 PSUM banks)
MMCOLS = 512                # moving-operand max
N_CORES = 8
NHALF = ROW_TILES * 2       # 64

_NC_CACHE = {}


def _argmax_ref(in0, in1, c0, c1, c2):
    r = np.maximum.accumulate(in0.astype(np.float32), axis=1)
    idx = np.arange(in0.shape[1], dtype=np.float32)[None, :]
    out = np.where(in0.astype(np.float32) >= r, idx, -np.finfo(np.float32).max)
    acc = out.max(axis=1, keepdims=True)
    return out, acc


def _get_argmax_op():
    """Register the fused argmax custom DVE op (idempotent)."""
    name = "ARGMAX_LAST_ANT"
    for op in dve_ops.OPS:
        if op.name == name:
            return op
    spec = Spec(
        body=select(Src0 >= scan(AluOp.MAX, Src0), Idx, MaxNeg),
        accum=AluOp.MAX,
        reference=_argmax_ref,
    )
    row = dve_ops._CUSTOM_DVE_ROW_BASE + len(dve_ops.OPS)
    dve_ops._SUB_OPCODE_FOR_NAME[name] = row
    shas = {
        ver: DveOpSpec(name=name, opcode=row, uops=lower(spec, ver=ver),
                       rd1_en=False).sha(ver)
        for ver in ("v3", "v4")
    }
    op = DveOp(name, spec, subdim=False, uops_sha=shas)
    dve_ops.OPS.append(op)
    dve_ops.CUSTOM_DVE_SPECS[name] = spec
    return op


def _build_nc(repeat=1):
    """Build the SPMD bass program (identical on all 8 cores)."""
    key = ("nc", repeat)
    if key in _NC_CACHE:
        return _NC_CACHE[key]

    argmax_op = _get_argmax_op()

    nc = bacc.Bacc("TRN2", target_bir_lowering=False)
    f32 = mybir.dt.float32
    f32r = mybir.dt.float32r

    lhsT_d = nc.dram_tensor("lhsT", [K, N], f32r, kind="ExternalInput")
    rhs_d = nc.dram_tensor("rhs", [K, N], f32r, kind="ExternalInput")
    idxs_d = nc.dram_tensor("idxs", [P, NHALF], f32, kind="ExternalOutput")

    with tile.TileContext(nc) as tc, ExitStack() as ctx:
        inp = ctx.enter_context(tc.tile_pool(name="inp", bufs=1))
        stage = ctx.enter_context(tc.tile_pool(name="stage", bufs=1))
        psum = ctx.enter_context(tc.tile_pool(name="psum", bufs=2, space="PSUM"))

        lhsT_s = inp.tile([K, N], f32r)
        rhs_s = inp.tile([K, N], f32r)
        nc.sync.dma_start(lhsT_s[:], lhsT_d[:])
        nc.sync.dma_start(rhs_s[:], rhs_d[:])

        istage = stage.tile([P, NHALF], f32)
        scratch = stage.tile([P, HALF], f32)

        for t in range(ROW_TILES * repeat):
            rt = t % ROW_TILES
            for h in range(2):
                i = rt * 2 + h
                pt = psum.tile([P, HALF], f32)
                if t == 0 and h == 0:
                    # Dummy matmul reading only rhs_s: the PE weight-load HW
                    # slot carries a single semaphore wait, so the two input
                    # DMA waits must land on separate PE instructions.
                    nc.tensor.matmul(
                        pt[:, 0:MMCOLS], rhs_s[:, 0:P], rhs_s[:, 0:MMCOLS],
                        start=True, stop=True,
                    )
                for j in range(HALF // MMCOLS):
                    nc.tensor.matmul(
                        pt[:, bass.ts(j, MMCOLS)],
                        lhsT_s[:, bass.ts(rt, P)],
                        rhs_s[:, h * HALF + j * MMCOLS
                              : h * HALF + (j + 1) * MMCOLS],
                        start=True,
                        stop=True,
                    )
                # One fused pass straight from PSUM:
                #   istage[:, i] = argmax_m(pt[:, m])  (last occurrence)
                nc.vector._custom_dve(
                    argmax_op, out=scratch[:], in0=pt[:],
                    accum_out=istage[:, i : i + 1],
                )

        nc.sync.dma_start(idxs_d[:], istage[:])

    nc.finalize()
    _NC_CACHE[key] = nc
    return nc


def _get_runner(repeat=1):
    """Build the sharded PJRT executable once; reuse across kernel() calls."""
    rkey = ("runner", repeat)
    if rkey in _NC_CACHE:
        return _NC_CACHE[rkey]
    run = _make_runner_from_nc(_build_nc(repeat))
    _NC_CACHE[rkey] = run
    return run


def _make_runner_from_nc(nc):
    import jax
    from jax.sharding import Mesh, PartitionSpec
    from jax.experimental.shard_map import shard_map
    from concourse import bass2jax

    bass2jax.install_neuronx_cc_hook()

    partition_name = nc.partition_id_tensor.name if nc.partition_id_tensor else None
    in_names, out_names, out_avals, zero_outs = [], [], [], []
    for alloc in nc.m.functions[0].allocations:
        if not isinstance(alloc, mybir.MemoryLocationSet):
            continue
        name = alloc.memorylocations[0].name
        if alloc.kind == "ExternalInput":
            if name != partition_name:
                in_names.append(name)
        elif alloc.kind == "ExternalOutput":
            shape = tuple(alloc.tensor_shape)
            np_dt = mybir.dt.np(alloc.dtype)
            out_names.append(name)
            out_avals.append(jax.core.ShapedArray(shape, np_dt))
            zero_outs.append(np.zeros(shape, np_dt))

    n_params = len(in_names)
    n_outs = len(out_names)
    all_in_names = list(in_names) + list(out_names)
    if partition_name is not None:
        all_in_names.append(partition_name)
    donate = tuple(range(n_params, n_params + n_outs))

    def _body(*args):
        operands = list(args)
        if partition_name is not None:
            operands.append(bass2jax.partition_id_tensor())
        outs = bass2jax._bass_exec_p.bind(
            *operands,
            out_avals=tuple(out_avals),
            in_names=tuple(all_in_names),
            out_names=tuple(out_names),
            lowering_input_output_aliases=(),
            sim_require_finite=True,
            sim_require_nnan=True,
            nc=nc,
        )
        return tuple(outs)

    devices = jax.devices()[:N_CORES]
    mesh = Mesh(np.asarray(devices), ("core",))
    in_specs = (PartitionSpec("core"),) * (n_params + n_outs)
    out_specs = (PartitionSpec("core"),) * n_outs
    sharded = jax.jit(
        shard_map(_body, mesh=mesh, in_specs=in_specs, out_specs=out_specs,
                  check_rep=False),
        donate_argnums=donate, keep_unused=True,
    )

    def run(in_maps):
        concat_in = [
            np.concatenate([np.asarray(m[name]) for m in in_maps], axis=0)
            for name in in_names
        ]
        concat_zeros = [
            np.zeros((N_CORES * z.shape[0], *z.shape[1:]), z.dtype)
            for z in zero_outs
        ]
        out_arrs = sharded(*concat_in, *concat_zeros)
        return [
            {
                name: np.asarray(out_arrs[k]).reshape(
                    N_CORES, *out_avals[k].shape)[c]
                for k, name in enumerate(out_names)
            }
            for c in range(N_CORES)
        ]

    return run


# ---------------- host-side numpy port of the tiny reference pieces ----------


def _normalize(x, axis, eps=EPS):
    n = np.linalg.norm(x, axis=axis, keepdims=True)
    return x / np.maximum(n, eps)


def _skew(k):
    kx, ky, kz = k[:, 0], k[:, 1], k[:, 2]
    O = np.zeros_like(kx)
    row0 = np.stack([O, -kz, ky], axis=1)
    row1 = np.stack([kz, O, -kx], axis=1)
    row2 = np.stack([-ky, kx, O], axis=1)
    return np.stack([row0, row1, row2], axis=1)


def _gravity_align(g_src, g_tgt, eps=EPS):
    u = _normalize(g_src, 1, eps)
    v = _normalize(g_tgt, 1, eps)
    axis = np.cross(u, v)
    axis_norm = np.linalg.norm(axis, axis=1, keepdims=True)
    dot = np.clip(np.sum(u * v, axis=1, keepdims=True), -1.0, 1.0)
    parallel = axis_norm < 1e-6
    k = axis / (axis_norm + eps)
    theta = np.arccos(dot)
    sin_t, cos_t = np.sin(theta), np.cos(theta)
    Kk = _skew(k)
    I = np.eye(3, dtype=g_src.dtype)[None]
    R = I + sin_t[:, :, None] * Kk + (1.0 - cos_t)[:, :, None] * (Kk @ Kk)
    ex = np.array([1.0, 0.0, 0.0], dtype=u.dtype)[None]
    ey = np.array([0.0, 1.0, 0.0], dtype=u.dtype)[None]
    use_ex = np.abs(u[:, 0:1]) < 0.9
    basis = np.where(use_ex, ex, ey)
    axis2 = _normalize(np.cross(u, basis), 1, eps)
    K2 = _skew(axis2)
    R_anti = I + 2.0 * (K2 @ K2)
    antipar = parallel & (dot < 0.0)
    R = np.where(antipar[:, :, None], R_anti, R)
    R = np.where((parallel & (dot > 0.0))[:, :, None], I, R)
    return R.astype(np.float32)


def _decode_core(idxs):
    """[P, 64] staging -> (idx0[4096], idx1[4096]) candidate indices.

    Row r = 128*t + p lives at partition p, column 2t+h.  idx1 is relative
    to the second half (add HALF for the global column).
    """
    i = np.rint(idxs.reshape(P, ROW_TILES, 2)).astype(np.int64)
    i0 = np.clip(i[:, :, 0], 0, HALF - 1)
    i1 = np.clip(i[:, :, 1], 0, HALF - 1) + HALF
    return i0.T.reshape(N), i1.T.reshape(N)


def _sigmoid(x):
    out = np.empty_like(x)
    pos = x >= 0
    out[pos] = 1.0 / (1.0 + np.exp(-x[pos]))
    ex = np.exp(x[~pos])
    out[~pos] = ex / (1.0 + ex)
    return out


def _host_prep(src, tgt, src_n, tgt_n, g_p, k_p, g_q, k_q):
    src = np.asarray(src, np.float32)
    tgt = np.asarray(tgt, np.float32)
    src_n = np.asarray(src_n, np.float32)
    g_p = np.asarray(g_p, np.float32)
    g_q = np.asarray(g_q, np.float32)

    R_g = _gravity_align(g_p, g_q)
    src_rot = np.einsum("bij,bjn->bin", R_g, src).astype(np.float32)
    src_n_rot = np.einsum("bij,bjn->bin", R_g, src_n).astype(np.float32)
    t_center = tgt.mean(axis=2, keepdims=True) - src_rot.mean(axis=2, keepdims=True)
    s = (src_rot + t_center).astype(np.float32)  # src_init

    xx = np.sum(s * s, axis=1)  # [B, N]
    yy = np.sum(tgt * tgt, axis=1)
    return s, tgt, src_n_rot, xx, yy


def prepare_in_maps(src, tgt, src_n, tgt_n, g_p, k_p, g_q, k_q):
    s, tgt, _, xx, yy = _host_prep(src, tgt, src_n, tgt_n, g_p, k_p, g_q, k_q)
    ones = np.ones((1, N), np.float32)
    in_maps = []
    for c in range(N_CORES):
        b, o = c % B, c // B
        if o == 0:
            lhsT = np.concatenate([2.0 * s[b], xx[b][None], ones], axis=0)
            rhs = np.concatenate([tgt[b], -ones, -yy[b][None]], axis=0)
        else:
            lhsT = np.concatenate([2.0 * tgt[b], yy[b][None], ones], axis=0)
            rhs = np.concatenate([s[b], -ones, -xx[b][None]], axis=0)
        in_maps.append(
            {"lhsT": np.ascontiguousarray(lhsT, np.float32),
             "rhs": np.ascontiguousarray(rhs, np.float32)}
        )
    return in_maps


def _combine_halves(A, Bm, idx0, idx1):
    """Pick the better of the two half-candidates using exact distances.

    A [3,N] query points, Bm [3,N] reference points; idx0/idx1 [N] global
    candidate indices.  Returns (min_d2 [N], corr [N]).
    """
    d0 = ((A - Bm[:, idx0]) ** 2).sum(axis=0)
    d1 = ((A - Bm[:, idx1]) ** 2).sum(axis=0)
    take1 = d1 < d0
    return np.where(take1, d1, d0).astype(np.float32), np.where(take1, idx1, idx0)


def kernel(src, tgt, src_n, tgt_n, g_p, k_p, g_q, k_q):
    src = np.asarray(src, np.float32)
    tgt = np.asarray(tgt, np.float32)
    src_n = np.asarray(src_n, np.float32)
    tgt_n = np.asarray(tgt_n, np.float32)
    g_p = np.asarray(g_p, np.float32)
    g_q = np.asarray(g_q, np.float32)
    k_p = np.asarray(k_p, np.float32)
    k_q = np.asarray(k_q, np.float32)

    s, tgt, src_n_rot, xx, yy = _host_prep(
        src, tgt, src_n, tgt_n, g_p, k_p, g_q, k_q)
    in_maps = prepare_in_maps(src, tgt, src_n, tgt_n, g_p, k_p, g_q, k_q)

    results = _get_runner()(in_maps)

    min_pq = np.empty((B, N), np.float32)
    corr_p2q = np.empty((B, N), np.int64)
    min_qp = np.empty((B, N), np.float32)
    corr_q2p = np.empty((B, N), np.int64)
    for c in range(N_CORES):
        b, o = c % B, c // B
        idx0, idx1 = _decode_core(results[c]["idxs"])
        if o == 0:
            min_pq[b], corr_p2q[b] = _combine_halves(s[b], tgt[b], idx0, idx1)
        else:
            min_qp[b], corr_q2p[b] = _combine_halves(tgt[b], s[b], idx0, idx1)

    nn_d_p = np.sqrt(np.maximum(min_pq, 0.0) + EPS)
    nn_d_q = np.sqrt(np.maximum(min_qp, 0.0) + EPS)
    tau_p = DIST_SCALE * np.sort(nn_d_p, axis=1)[:, (N - 1) // 2][:, None]
    tau_q = DIST_SCALE * np.sort(nn_d_q, axis=1)[:, (N - 1) // 2][:, None]
    geom_p = (nn_d_p <= tau_p).astype(np.float32)
    geom_q = (nn_d_q <= tau_q).astype(np.float32)

    gq = g_q[:, :, None]
    inc_p = np.sum(src_n_rot * gq, axis=1)  # [B, N]
    inc_q = np.sum(tgt_n * gq, axis=1)
    inc_p_ref = np.take_along_axis(inc_q, corr_p2q, axis=1)
    inc_q_ref = np.take_along_axis(inc_p, corr_q2p, axis=1)

    k_eff = k_p * k_q / (k_p + k_q + EPS)  # [B,1]
    w_p = _sigmoid(CHI2_THRESH - k_eff * (inc_p - inc_p_ref) ** 2) * geom_p
    w_q = _sigmoid(CHI2_THRESH - k_eff * (inc_q - inc_q_ref) ** 2) * geom_q
    return w_p[:, None, :].astype(np.float32), w_q[:, None, :].astype(np.float32)
